# revision 18
# baseline (speedup 1.0000x reference)
"""AttentionBlock (GroupNorm -> 1x1-conv QKV -> softmax attention -> 1x1-conv proj
-> residual) for Trainium2, data-parallel over batch across 8 NeuronCores.

Shapes (hardcoded): x [B=8, C=64, H=64, W=64] fp32; N = H*W = 4096.
Each core processes one sample end-to-end; no cross-core communication.

Per-core algorithm (C=64 channels on partitions, N=4096 spatial on free dim):
  1. GroupNorm(8 groups): per-channel bn_stats/bn_aggr -> tiny matmuls with
     group masks to reduce/broadcast across the 8-channel groups -> fused
     per-partition affine h = x*a + b.
  2. q = (Wq/8) h + bq/8, k = Wk h  (bk dropped: it shifts every score in a
     softmax row by a constant -> softmax-invariant). v computed directly in
     transposed [N, C] layout, augmented with a ones column so the attention
     matmul also produces the softmax denominator.
  3. Scores computed transposed, sT[m, n] = sum_c k[c,m] q[c,n], in [128, 512]
     PSUM tiles; exp on ScalarE straight PSUM->SBUF (no row-max subtraction:
     scores are O(1) here, fp32 exp is exact enough); AV matmul accumulates
     out[c, n] (+ den[n] in row 64) over the 32 m-chunks.
  4. proj = Wp @ out_unnormalized, then multiply by 1/den (column scaling
     commutes with the left matmul), add bp' = bp + Wp bv (bv folded: rows of
     attn sum to 1), add residual x.
"""

import os
import numpy as np

import concourse.bass as bass
import concourse.bacc as bacc
import concourse.mybir as mybir
from concourse.tile import TileContext
from concourse.bass_utils import run_bass_kernel_spmd

FP = mybir.dt.float32
B, C, H, W = 8, 64, 64, 64
N = H * W          # 4096
G = 8              # groups
NT = 512           # n-tile (free dim of score tiles)
MT = 128           # m-tile (partition dim of score tiles)
N_NT = N // NT     # 8
N_MT = N // MT     # 32
EPS = 1e-5

# exp group: how many m-tiles share one PSUM region / one ACT exp instruction
EXPG = 2           # [128, EXPG*512] psum tiles

last_run_info = {}


def build_program(debug=False):
    # Bacc (not raw Bass): its finalize pipeline splits multi-sem waits
    # (fp32 self-loading matmuls only support a single sync wait).
    nc = bacc.Bacc()
    dbg = {}
    if debug:
        for nm, shp in [("dbg_h", [C, N]), ("dbg_q", [C, N]), ("dbg_k", [C, N]),
                        ("dbg_vt", [128, N_MT * (C + 1)]),
                        ("dbg_av", [C, N]), ("dbg_den", [1, N]),
                        ("dbg_denr", [1, N]), ("dbg_dbc", [C, N])]:
            dbg[nm] = nc.dram_tensor(nm, shp, FP, kind="ExternalOutput")

    x_d = nc.dram_tensor("x", [C, N], FP, kind="ExternalInput")
    wqT2_d = nc.dram_tensor("wqT", [C, C], FP, kind="ExternalInput")   # Wq.T/8
    bq_d = nc.dram_tensor("bq", [C, 1], FP, kind="ExternalInput")      # bq/8
    wkT_d = nc.dram_tensor("wkT", [C, C], FP, kind="ExternalInput")    # Wk.T
    wvT_d = nc.dram_tensor("wvT", [C, C], FP, kind="ExternalInput")    # Wv.T
    wpT_d = nc.dram_tensor("wpT", [C, C], FP, kind="ExternalInput")    # Wp.T
    bpp_d = nc.dram_tensor("bpp", [C, 1], FP, kind="ExternalInput")    # bp + Wp@bv
    gamma_d = nc.dram_tensor("gamma", [C, 1], FP, kind="ExternalInput")
    beta_d = nc.dram_tensor("beta", [C, 1], FP, kind="ExternalInput")
    gmask_d = nc.dram_tensor("gmask", [C, G], FP, kind="ExternalInput")   # 1/8 blocks
    gbcast_d = nc.dram_tensor("gbcast", [G, C], FP, kind="ExternalInput") # 1 blocks
    out_d = nc.dram_tensor("out", [C, N], FP, kind="ExternalOutput")

    with TileContext(nc) as tc:
        with (
            tc.tile_pool(name="const", bufs=1) as const,
            tc.tile_pool(name="big", bufs=1) as big,
            tc.tile_pool(name="epool", bufs=4) as epool,
            tc.tile_pool(name="small", bufs=4) as small,
            tc.tile_pool(name="outp", bufs=3) as outp,
            tc.tile_pool(name="qk_ps", bufs=2, space="PSUM") as qk_ps,
            tc.tile_pool(name="av_ps", bufs=2, space="PSUM") as av_ps,
            tc.tile_pool(name="post_ps", bufs=2, space="PSUM") as post_ps,
        ):
            # ---- constant loads ----
            wqT2 = const.tile([C, C], FP, tag="wqT2")
            wkT = const.tile([C, C], FP, tag="wkT")
            wvT = const.tile([C, C], FP, tag="wvT")
            wpT = const.tile([C, C], FP, tag="wpT")
            bq = const.tile([C, 1], FP, tag="bq")
            bpp = const.tile([C, 1], FP, tag="bpp")
            gamma = const.tile([C, 1], FP, tag="gamma")
            beta = const.tile([C, 1], FP, tag="beta")
            gmask = const.tile([C, G], FP, tag="gmask")
            gbcast = const.tile([G, C], FP, tag="gbcast")
            for t, d in [(bq, bq_d), (bpp, bpp_d),
                         (gamma, gamma_d), (beta, beta_d)]:
                nc.sync.dma_start(out=t[:], in_=d[:])
            # fp32 matmuls lower to a self-loading LDWEIGHTS that supports
            # only ONE sync wait; matmul operands coming straight off DMA
            # would need DMA+DVE waits. Funnel them through a DVE copy so
            # every matmul dep collapses onto the DVE semaphore.
            for t, d in [(wqT2, wqT2_d), (wkT, wkT_d), (wvT, wvT_d),
                         (wpT, wpT_d), (gmask, gmask_d), (gbcast, gbcast_d)]:
                stg = small.tile(list(t.shape), FP, tag="const_stage")
                nc.sync.dma_start(out=stg[:], in_=d[:])
                nc.vector.tensor_copy(out=t[:], in_=stg[:])

            ones_col = const.tile([128, C], FP, tag="ones_col")
            nc.vector.memset(ones_col[:], 1.0)
            eps_sb = const.tile([128, 1], FP, tag="eps")
            nc.vector.memset(eps_sb[:], EPS)

            # ---- load x ----
            x_sb = big.tile([C, N], FP, tag="x")
            nc.sync.dma_start(out=x_sb[:], in_=x_d[:])

            # ---- GroupNorm ----
            stats = small.tile([C, N // 512, 6], FP, tag="gn_stats")
            for j in range(N // 512):
                nc.vector.bn_stats(out=stats[:, j, :], in_=x_sb[:, j * 512:(j + 1) * 512])
            mv = small.tile([C, 2], FP, tag="gn_mv")
            nc.vector.bn_aggr(out=mv[:], in_=stats[:])
            # mm2 = [mean_c, mean_c^2 + var_c]
            mm2 = small.tile([C, 2], FP, tag="gn_mm2")
            nc.vector.tensor_copy(out=mm2[:, 0:1], in_=mv[:, 0:1])
            t0 = small.tile([C, 1], FP, tag="gn_t0")
            nc.vector.tensor_mul(out=t0[:], in0=mv[:, 0:1], in1=mv[:, 0:1])
            nc.vector.tensor_add(out=mm2[:, 1:2], in0=t0[:], in1=mv[:, 1:2])
            # group stats: [G, 2] = gmask.T @ mm2   (gmask holds 1/8)
            gstat_ps = post_ps.tile([128, 512], FP, tag="post")
            nc.tensor.matmul(out=gstat_ps[0:G, 0:2], lhsT=gmask[:], rhs=mm2[:])
            gstat = small.tile([G, 2], FP, tag="gn_gstat")
            nc.vector.tensor_copy(out=gstat[:], in_=gstat_ps[0:G, 0:2])
            # var_g = E[x^2]_g - mean_g^2 ; rstd = 1/sqrt(var+eps)
            vg = small.tile([G, 1], FP, tag="gn_vg")
            nc.vector.tensor_mul(out=vg[:], in0=gstat[:, 0:1], in1=gstat[:, 0:1])
            nc.vector.tensor_sub(out=vg[:], in0=gstat[:, 1:2], in1=vg[:])
            stdg = small.tile([G, 1], FP, tag="gn_stdg")
            nc.scalar.activation(out=stdg[:], in_=vg[:],
                                 func=mybir.ActivationFunctionType.Sqrt,
                                 bias=eps_sb[0:G, :])
            rhs2 = small.tile([G, 2], FP, tag="gn_rhs2")
            nc.vector.tensor_copy(out=rhs2[:, 0:1], in_=gstat[:, 0:1])
            nc.vector.reciprocal(out=rhs2[:, 1:2], in_=stdg[:])
            # broadcast to channels: [C, 2] = gbcast.T @ rhs2
            pstat_ps = post_ps.tile([128, 512], FP, tag="post")
            nc.tensor.matmul(out=pstat_ps[0:C, 0:2], lhsT=gbcast[:], rhs=rhs2[:])
            a_sb = small.tile([C, 1], FP, tag="gn_a")
            b_sb = small.tile([C, 1], FP, tag="gn_b")
            nc.vector.tensor_mul(out=a_sb[:], in0=pstat_ps[0:C, 1:2], in1=gamma[:])
            nc.vector.tensor_mul(out=b_sb[:], in0=pstat_ps[0:C, 0:1], in1=a_sb[:])
            nc.vector.tensor_sub(out=b_sb[:], in0=beta[:], in1=b_sb[:])
            h_sb = big.tile([C, N], FP, tag="h")
            nc.vector.tensor_scalar(out=h_sb[:], in0=x_sb[:],
                                    scalar1=a_sb[:], scalar2=b_sb[:],
                                    op0=mybir.AluOpType.mult,
                                    op1=mybir.AluOpType.add)

            # ---- QKV projections ----
            q_sb = big.tile([C, N], FP, tag="q")
            k_sb = big.tile([C, N], FP, tag="k")
            for j in range(N_NT):
                sl = slice(j * NT, (j + 1) * NT)
                qp = qk_ps.tile([128, EXPG * NT], FP, tag="qk")
                nc.tensor.matmul(out=qp[0:C, 0:NT], lhsT=wqT2[:], rhs=h_sb[:, sl])
                nc.vector.tensor_scalar_add(out=q_sb[:, sl], in0=qp[0:C, 0:NT], scalar1=bq[:])
                kp = qk_ps.tile([128, EXPG * NT], FP, tag="qk")
                nc.tensor.matmul(out=kp[0:C, 0:NT], lhsT=wkT[:], rhs=h_sb[:, sl])
                nc.vector.tensor_copy(out=k_sb[:, sl], in_=kp[0:C, 0:NT])

            if debug:
                nc.sync.dma_start(out=dbg["dbg_h"][:], in_=h_sb[:])
                nc.sync.dma_start(out=dbg["dbg_q"][:], in_=q_sb[:])
                nc.sync.dma_start(out=dbg["dbg_k"][:], in_=k_sb[:])

            # vT_aug[p, mt, 0:64] = v[m = mt*128+p, c]; vT_aug[p, mt, 64] = 1
            vT = big.tile([128, N_MT, C + 1], FP, tag="vT")
            nc.vector.memset(vT[:, :, C:C + 1], 1.0)
            for mt in range(0, N_MT, 4):
                vp = av_ps.tile([128, NT], FP, tag="av")
                for j in range(4):
                    nc.tensor.matmul(out=vp[:, j * C:(j + 1) * C],
                                     lhsT=h_sb[:, (mt + j) * MT:(mt + j + 1) * MT],
                                     rhs=wvT[:])
                nc.vector.tensor_copy(
                    out=vT[:, mt:mt + 4, 0:C],
                    in_=vp[:, 0:4 * C].rearrange("p (j c) -> p j c", j=4))

            if debug:
                nc.sync.dma_start(
                    out=dbg["dbg_vt"][:],
                    in_=vT[:].rearrange("p a b -> p (a b)"))

            # ---- attention ----
            for nt in range(N_NT):
                nsl = slice(nt * NT, (nt + 1) * NT)
                av = av_ps.tile([128, NT], FP, tag="av")
                e_tiles = []
                for g in range(N_MT // EXPG):
                    sp = qk_ps.tile([128, EXPG * NT], FP, tag="qk")
                    for j in range(EXPG):
                        mt = g * EXPG + j
                        nc.tensor.matmul(
                            out=sp[:, j * NT:(j + 1) * NT],
                            lhsT=k_sb[:, mt * MT:(mt + 1) * MT],
                            rhs=q_sb[:, nsl])
                    e = epool.tile([128, EXPG * NT], FP, tag="e")
                    nc.scalar.activation(out=e[:], in_=sp[:],
                                         func=mybir.ActivationFunctionType.Exp)
                    e_tiles.append((g, e))
                    # attention-value matmuls (accumulate over all m chunks)
                    for j in range(EXPG):
                        mt = g * EXPG + j
                        nc.tensor.matmul(
                            out=av[0:C + 1, :],
                            lhsT=vT[:, mt, :],
                            rhs=e[:, j * NT:(j + 1) * NT],
                            start=(mt == 0), stop=(mt == N_MT - 1),
                            skip_group_check=True)

                # denominator reciprocal (row 64 of av)
                denr = small.tile([128, NT], FP, tag="denr")
                nc.vector.reciprocal(out=denr[C:C + 1, :], in_=av[C:C + 1, :])
                # broadcast 1/den across 64 partitions via K=1 matmul
                dbc_ps = post_ps.tile([128, 512], FP, tag="post")
                nc.tensor.matmul(out=dbc_ps[0:C, :], lhsT=ones_col[C:C + 1, :],
                                 rhs=denr[C:C + 1, :])
                dbc = outp.tile([C, NT], FP, tag="dbc")
                nc.vector.tensor_copy(out=dbc[:], in_=dbc_ps[0:C, :])
                if debug:
                    nc.sync.dma_start(out=dbg["dbg_denr"][:, nsl], in_=denr[C:C + 1, :])
                    nc.sync.dma_start(out=dbg["dbg_dbc"][:, nsl], in_=dbc[:])
                # unnormalized attention output -> SBUF for proj matmul
                av_sb = outp.tile([C, NT], FP, tag="av_sb")
                nc.vector.tensor_copy(out=av_sb[:], in_=av[0:C, :])
                if debug:
                    den_sb = outp.tile([128, NT], FP, tag="den_dbg")
                    nc.vector.tensor_copy(out=den_sb[C:C + 1, :], in_=av[C:C + 1, :])
                    nc.sync.dma_start(out=dbg["dbg_den"][:, nsl], in_=den_sb[C:C + 1, :])
                    nc.sync.dma_start(out=dbg["dbg_av"][:, nsl], in_=av_sb[:])
                # proj
                pj_ps = post_ps.tile([128, 512], FP, tag="post")
                nc.tensor.matmul(out=pj_ps[0:C, :], lhsT=wpT[:], rhs=av_sb[:])
                o_sb = outp.tile([C, NT], FP, tag="o_sb")
                nc.vector.tensor_mul(out=o_sb[:], in0=pj_ps[0:C, :], in1=dbc[:])
                nc.vector.scalar_tensor_tensor(
                    out=o_sb[:], in0=o_sb[:], scalar=bpp[:], in1=x_sb[:, nsl],
                    op0=mybir.AluOpType.add, op1=mybir.AluOpType.add)
                nc.sync.dma_start(out=out_d[:, nsl], in_=o_sb[:])

    nc.finalize()  # Bacc.finalize runs the wait-splitting legalization
    return nc


_cached = {}


def _install_trace_hook():
    """The agent image lacks antenv.axon_hooks, so run_bass_kernel_spmd's
    trace path degrades. Recreate the module + NTFF hook locally."""
    import sys, types
    import antenv
    if "antenv.axon_hooks" in sys.modules:
        return
    mod = types.ModuleType("antenv.axon_hooks")
    holder = {"hook": None}
    mod.set_axon_ntff_profile_hook = lambda h: holder.__setitem__("hook", h)
    mod.get_axon_ntff_profile_hook = lambda: holder["hook"]
    sys.modules["antenv.axon_hooks"] = mod
    antenv.axon_hooks = mod
    from trn_agent_boot.trn_boot import _ntff_profile_via_ctypes
    mod.set_axon_ntff_profile_hook(_ntff_profile_via_ctypes("/opt/axon/libaxon_pjrt.so"))
    import concourse.bass_utils as bu
    bu.upload_artifacts = lambda tmpdir: tmpdir


def kernel(x, gn_w, gn_b, Wq, bq, Wk, bk, Wv, bv, Wp, bp, _trace=False):
    x = np.ascontiguousarray(np.asarray(x, np.float32)).reshape(B, C, N)
    f32 = lambda a: np.ascontiguousarray(np.asarray(a, np.float32))
    Wq, Wk, Wv, Wp = f32(Wq), f32(Wk), f32(Wv), f32(Wp)
    bq, bk, bv, bp = f32(bq), f32(bk), f32(bv), f32(bp)

    scale = 1.0 / np.sqrt(np.float32(C))
    gmask = np.zeros((C, G), np.float32)
    gbcast = np.zeros((G, C), np.float32)
    for g in range(G):
        gmask[g * 8:(g + 1) * 8, g] = 1.0 / 8.0
        gbcast[g, g * 8:(g + 1) * 8] = 1.0
    consts = {
        "wqT": f32(Wq.T * scale),
        "bq": f32(bq * scale)[:, None],
        "wkT": f32(Wk.T),
        "wvT": f32(Wv.T),
        "wpT": f32(Wp.T),
        "bpp": f32(bp + Wp @ bv)[:, None],
        "gamma": f32(gn_w)[:, None],
        "beta": f32(gn_b)[:, None],
        "gmask": gmask,
        "gbcast": gbcast,
    }

    if _trace:
        _install_trace_hook()

    if "nc" not in _cached:
        _cached["nc"] = build_program()
    nc = _cached["nc"]

    in_maps = [dict(consts, x=np.ascontiguousarray(x[i])) for i in range(B)]
    res = run_bass_kernel_spmd(nc, in_maps, core_ids=list(range(B)), trace=_trace)
    last_run_info["exec_time_ns"] = res.exec_time_ns
    last_run_info["mean_exec_time_ns"] = res.mean_exec_time_ns
    out = np.stack([res.results[i]["out"] for i in range(B)], axis=0)
    return out.reshape(B, C, H, W)


# revision 19
# speedup vs baseline: 2.4860x; 2.4860x over previous
"""AttentionBlock (GroupNorm -> 1x1-conv QKV -> softmax attention -> 1x1-conv proj
-> residual) for Trainium2, data-parallel over batch across 8 NeuronCores.

Shapes (hardcoded): x [B=8, C=64, H=64, W=64] fp32; N = H*W = 4096.
Each core processes one sample end-to-end; no cross-core communication.

Per-core algorithm (C=64 channels on partitions, N=4096 spatial on free dim):
  1. GroupNorm(8 groups): per-channel bn_stats/bn_aggr -> tiny matmuls with
     group masks to reduce/broadcast across the 8-channel groups -> fused
     per-partition affine h = x*a + b.
  2. q = (Wq/8) h + bq/8, k = Wk h (bk dropped: it shifts every score in a
     softmax row by a constant -> softmax-invariant). q,k are produced
     duplicated on both partition halves ([128, N] via stacked weights) and
     cast to fp16 so the score matmuls can run 2-at-a-time in the PE's
     64x128 row-tiling mode. v is computed in transposed [N, C] layout,
     augmented with a ones column so the attention-value matmul also
     accumulates the softmax denominator.
  3. Scores computed transposed, sT[m, n] = sum_c k[c,m] q[c,n], in fp16
     (fp32 matmuls cost two PE passes; fp16 is one, and scores are O(1) so
     fp16 keeps ~3 significant digits -> output error ~1e-4). exp on ScalarE
     straight PSUM->SBUF (no row-max subtraction needed). AV matmul (fp16)
     accumulates out[c, n] (+ den[n] in psum row 64) over the 32 m-chunks.
  4. proj = Wp @ out_unnormalized (fp32), then multiply by 1/den (column
     scaling commutes with the left matmul), add bp' = bp + Wp bv (bv folds:
     attn rows sum to 1), add residual x.

The nt loop is software-pipelined: scores/exp for tile nt are emitted before
the AV/postprocessing of tile nt-1, so the PE fills ScalarE's shadow and the
ScalarE exp stream (the roofline engine at ~1 elem/lane/cycle) never starves.
"""

import os
import numpy as np

import concourse.bass as bass
import concourse.bacc as bacc
import concourse.mybir as mybir
from concourse.tile import TileContext
from concourse.bass_utils import run_bass_kernel_spmd

FP = mybir.dt.float32
F16 = mybir.dt.float16
B, C, H, W = 8, 64, 64, 64
N = H * W          # 4096
G = 8              # groups
NT = 512           # n-tile (free dim of score tiles)
MT = 128           # m-tile (partition dim of score tiles)
N_NT = N // NT     # 8
N_MT = N // MT     # 32
NPAIR = N_MT // 2  # 16 packed score-matmul pairs per n-tile
EPS = 1e-5

last_run_info = {}


def build_program(debug=False):
    # Bacc (not raw Bass): its finalize pipeline splits multi-sem waits
    # (fp32 self-loading matmuls only support a single sync wait).
    nc = bacc.Bacc()
    dbg = {}
    if debug:
        for nm, shp in [("dbg_h", [C, N]), ("dbg_q", [128, N]), ("dbg_k", [128, N]),
                        ("dbg_vt", [128, N_MT * (C + 1)]),
                        ("dbg_av", [C, N]), ("dbg_den", [1, N])]:
            dbg[nm] = nc.dram_tensor(nm, shp, FP, kind="ExternalOutput")

    x_d = nc.dram_tensor("x", [C, N], FP, kind="ExternalInput")
    wqT2_d = nc.dram_tensor("wqT2", [C, 2 * C], FP, kind="ExternalInput")  # [Wq.T|Wq.T]/8
    wkT2_d = nc.dram_tensor("wkT2", [C, 2 * C], FP, kind="ExternalInput")  # [Wk.T|Wk.T]
    bq2_d = nc.dram_tensor("bq2", [2 * C, 1], FP, kind="ExternalInput")    # tile(bq,2)/8
    wvT_d = nc.dram_tensor("wvT", [C, C], FP, kind="ExternalInput")        # Wv.T
    wpT_d = nc.dram_tensor("wpT", [C, C], FP, kind="ExternalInput")        # Wp.T
    bpp_d = nc.dram_tensor("bpp", [C, 1], FP, kind="ExternalInput")        # bp + Wp@bv
    gamma_d = nc.dram_tensor("gamma", [C, 1], FP, kind="ExternalInput")
    beta_d = nc.dram_tensor("beta", [C, 1], FP, kind="ExternalInput")
    gmask_d = nc.dram_tensor("gmask", [C, G], FP, kind="ExternalInput")    # 1/8 blocks
    gbcast_d = nc.dram_tensor("gbcast", [G, C], FP, kind="ExternalInput")  # 1 blocks
    out_d = nc.dram_tensor("out", [C, N], FP, kind="ExternalOutput")

    with TileContext(nc) as tc:
        with (
            tc.tile_pool(name="const", bufs=1) as const,
            tc.tile_pool(name="big", bufs=1) as big,
            tc.tile_pool(name="epool", bufs=2) as epool,
            tc.tile_pool(name="small", bufs=4) as small,
            tc.tile_pool(name="outp", bufs=3) as outp,
            tc.tile_pool(name="qk_ps", bufs=2, space="PSUM") as qk_ps,
            tc.tile_pool(name="av_ps", bufs=2, space="PSUM") as av_ps,
            tc.tile_pool(name="post_ps", bufs=2, space="PSUM") as post_ps,
        ):
            # ---- constant loads ----
            bq2 = const.tile([2 * C, 1], FP, tag="bq2")
            bpp = const.tile([C, 1], FP, tag="bpp")
            gamma = const.tile([C, 1], FP, tag="gamma")
            beta = const.tile([C, 1], FP, tag="beta")
            for t, d in [(bq2, bq2_d), (bpp, bpp_d), (gamma, gamma_d), (beta, beta_d)]:
                nc.sync.dma_start(out=t[:], in_=d[:])
            # fp32 matmuls lower to a self-loading LDWEIGHTS that supports
            # only ONE sync wait; matmul operands coming straight off DMA
            # would need DMA+DVE waits. Funnel them through a DVE copy so
            # every matmul dep collapses onto the DVE semaphore.
            wqT2 = const.tile([C, 2 * C], FP, tag="wqT2")
            wkT2 = const.tile([C, 2 * C], FP, tag="wkT2")
            wvT = const.tile([C, C], FP, tag="wvT")
            wpT = const.tile([C, C], FP, tag="wpT")
            gmask = const.tile([C, G], FP, tag="gmask")
            gbcast = const.tile([G, C], FP, tag="gbcast")
            for t, d in [(wqT2, wqT2_d), (wkT2, wkT2_d), (wvT, wvT_d),
                         (wpT, wpT_d), (gmask, gmask_d), (gbcast, gbcast_d)]:
                stg = small.tile(list(t.shape), FP, tag=f"stage_{t.shape[1]}")
                nc.sync.dma_start(out=stg[:], in_=d[:])
                nc.vector.tensor_copy(out=t[:], in_=stg[:])

            ones_col = const.tile([128, C], FP, tag="ones_col")
            nc.vector.memset(ones_col[:], 1.0)
            eps_sb = const.tile([128, 1], FP, tag="eps")
            nc.vector.memset(eps_sb[:], EPS)

            # ---- load x ----
            x_sb = big.tile([C, N], FP, tag="x")
            nc.sync.dma_start(out=x_sb[:], in_=x_d[:])

            # ---- GroupNorm ----
            stats = small.tile([C, N // 512, 6], FP, tag="gn_stats")
            for j in range(N // 512):
                nc.vector.bn_stats(out=stats[:, j, :], in_=x_sb[:, j * 512:(j + 1) * 512])
            mv = small.tile([C, 2], FP, tag="gn_mv")
            nc.vector.bn_aggr(out=mv[:], in_=stats[:])
            # mm2 = [mean_c, mean_c^2 + var_c]
            mm2 = small.tile([C, 2], FP, tag="gn_mm2")
            nc.vector.tensor_copy(out=mm2[:, 0:1], in_=mv[:, 0:1])
            t0 = small.tile([C, 1], FP, tag="gn_t0")
            nc.vector.tensor_mul(out=t0[:], in0=mv[:, 0:1], in1=mv[:, 0:1])
            nc.vector.tensor_add(out=mm2[:, 1:2], in0=t0[:], in1=mv[:, 1:2])
            # group stats: [G, 2] = gmask.T @ mm2   (gmask holds 1/8)
            gstat_ps = post_ps.tile([128, 512], FP, tag="post")
            nc.tensor.matmul(out=gstat_ps[0:G, 0:2], lhsT=gmask[:], rhs=mm2[:])
            gstat = small.tile([G, 2], FP, tag="gn_gstat")
            nc.vector.tensor_copy(out=gstat[:], in_=gstat_ps[0:G, 0:2])
            # var_g = E[x^2]_g - mean_g^2 ; rstd = 1/sqrt(var+eps)
            vg = small.tile([G, 1], FP, tag="gn_vg")
            nc.vector.tensor_mul(out=vg[:], in0=gstat[:, 0:1], in1=gstat[:, 0:1])
            nc.vector.tensor_sub(out=vg[:], in0=gstat[:, 1:2], in1=vg[:])
            stdg = small.tile([G, 1], FP, tag="gn_stdg")
            nc.scalar.activation(out=stdg[:], in_=vg[:],
                                 func=mybir.ActivationFunctionType.Sqrt,
                                 bias=eps_sb[0:G, :])
            rhs2 = small.tile([G, 2], FP, tag="gn_rhs2")
            nc.vector.tensor_copy(out=rhs2[:, 0:1], in_=gstat[:, 0:1])
            nc.vector.reciprocal(out=rhs2[:, 1:2], in_=stdg[:])
            # broadcast to channels: [C, 2] = gbcast.T @ rhs2
            pstat_ps = post_ps.tile([128, 512], FP, tag="post")
            nc.tensor.matmul(out=pstat_ps[0:C, 0:2], lhsT=gbcast[:], rhs=rhs2[:])
            a_sb = small.tile([C, 1], FP, tag="gn_a")
            b_sb = small.tile([C, 1], FP, tag="gn_b")
            nc.vector.tensor_mul(out=a_sb[:], in0=pstat_ps[0:C, 1:2], in1=gamma[:])
            nc.vector.tensor_mul(out=b_sb[:], in0=pstat_ps[0:C, 0:1], in1=a_sb[:])
            nc.vector.tensor_sub(out=b_sb[:], in0=beta[:], in1=b_sb[:])
            h_sb = big.tile([C, N], FP, tag="h")
            nc.vector.tensor_scalar(out=h_sb[:], in0=x_sb[:],
                                    scalar1=a_sb[:], scalar2=b_sb[:],
                                    op0=mybir.AluOpType.mult,
                                    op1=mybir.AluOpType.add)

            # ---- QKV projections (fp32 matmuls, fp16 outputs) ----
            # q2x/k2x: [128, N] with the channel block duplicated on both
            # partition halves, enabling 64x128 row-tiled score matmuls.
            q2x = big.tile([128, N], F16, tag="q2x")
            k2x = big.tile([128, N], F16, tag="k2x")
            for j in range(N_NT):
                sl = slice(j * NT, (j + 1) * NT)
                qp = qk_ps.tile([128, 2 * NT], FP, tag="qk")
                nc.tensor.matmul(out=qp[:, 0:NT], lhsT=wqT2[:], rhs=h_sb[:, sl])
                nc.tensor.matmul(out=qp[:, NT:2 * NT], lhsT=wkT2[:], rhs=h_sb[:, sl])
                nc.vector.tensor_scalar_add(out=q2x[:, sl], in0=qp[:, 0:NT], scalar1=bq2[:])
                nc.vector.tensor_copy(out=k2x[:, sl], in_=qp[:, NT:2 * NT])

            # vT_aug[p, mt, 0:64] = v[m = mt*128+p, c]; vT_aug[p, mt, 64] = 1
            vT = big.tile([128, N_MT, C + 1], F16, tag="vT")
            nc.vector.memset(vT[:, :, C:C + 1], 1.0)
            for mt in range(0, N_MT, 4):
                vp = av_ps.tile([128, NT], FP, tag="av")
                for j in range(4):
                    nc.tensor.matmul(out=vp[:, j * C:(j + 1) * C],
                                     lhsT=h_sb[:, (mt + j) * MT:(mt + j + 1) * MT],
                                     rhs=wvT[:])
                nc.vector.tensor_copy(
                    out=vT[:, mt:mt + 4, 0:C],
                    in_=vp[:, 0:4 * C].rearrange("p (j c) -> p j c", j=4))

            if debug:
                nc.sync.dma_start(out=dbg["dbg_h"][:], in_=h_sb[:])
                dq = big.tile([128, N], FP, tag="dbgq")
                dk = big.tile([128, N], FP, tag="dbgk")
                dv = big.tile([128, N_MT * (C + 1)], FP, tag="dbgv")
                nc.vector.tensor_copy(out=dq[:], in_=q2x[:])
                nc.vector.tensor_copy(out=dk[:], in_=k2x[:])
                nc.vector.tensor_copy(out=dv[:], in_=vT[:].rearrange("p a b -> p (a b)"))
                nc.sync.dma_start(out=dbg["dbg_q"][:], in_=dq[:])
                nc.sync.dma_start(out=dbg["dbg_k"][:], in_=dk[:])
                nc.sync.dma_start(out=dbg["dbg_vt"][:], in_=dv[:])

            # ---- attention (software-pipelined over n-tiles) ----
            e_tiles = {}
            av_tiles = {}

            def emit_scores(nt):
                nsl = slice(nt * NT, (nt + 1) * NT)
                e = epool.tile([128, NPAIR, 2 * NT], F16, tag="e")
                for p in range(NPAIR):
                    sp = qk_ps.tile([128, 2 * NT], FP, tag="qk")
                    mt_a, mt_b = 2 * p, 2 * p + 1
                    # row-tiled pair: T0 on partitions 0-63, T8 on 64-127
                    nc.tensor.matmul(out=sp[:, 0:NT],
                                     lhsT=k2x[0:C, mt_a * MT:(mt_a + 1) * MT],
                                     rhs=q2x[0:C, nsl])
                    nc.tensor.matmul(out=sp[:, NT:2 * NT],
                                     lhsT=k2x[C:2 * C, mt_b * MT:(mt_b + 1) * MT],
                                     rhs=q2x[C:2 * C, nsl])
                    nc.scalar.activation(out=e[:, p, :], in_=sp[:],
                                         func=mybir.ActivationFunctionType.Exp)
                e_tiles[nt] = e

            def emit_av_post(nt):
                nsl = slice(nt * NT, (nt + 1) * NT)
                e = e_tiles.pop(nt)
                av = av_ps.tile([128, NT], FP, tag="av")
                for p in range(NPAIR):
                    for j in range(2):
                        mt = 2 * p + j
                        nc.tensor.matmul(
                            out=av[0:C + 1, :],
                            lhsT=vT[:, mt, :],
                            rhs=e[:, p, j * NT:(j + 1) * NT],
                            start=(mt == 0), stop=(mt == N_MT - 1),
                            skip_group_check=True)
                # denominator reciprocal (psum row 64 of av)
                denr = small.tile([128, NT], FP, tag="denr")
                nc.vector.reciprocal(out=denr[C:C + 1, :], in_=av[C:C + 1, :])
                if debug:
                    den_sb = outp.tile([128, NT], FP, tag="den_dbg")
                    nc.vector.tensor_copy(out=den_sb[C:C + 1, :], in_=av[C:C + 1, :])
                    nc.sync.dma_start(out=dbg["dbg_den"][:, nsl], in_=den_sb[C:C + 1, :])
                # broadcast 1/den across 64 partitions via K=1 matmul
                dbc_ps = post_ps.tile([128, 512], FP, tag="post")
                nc.tensor.matmul(out=dbc_ps[0:C, :], lhsT=ones_col[C:C + 1, :],
                                 rhs=denr[C:C + 1, :])
                dbc = outp.tile([C, NT], FP, tag="dbc")
                nc.vector.tensor_copy(out=dbc[:], in_=dbc_ps[0:C, :])
                # unnormalized attention output -> SBUF for proj matmul
                av_sb = outp.tile([C, NT], FP, tag="av_sb")
                nc.vector.tensor_copy(out=av_sb[:], in_=av[0:C, :])
                if debug:
                    nc.sync.dma_start(out=dbg["dbg_av"][:, nsl], in_=av_sb[:])
                # proj, then scale columns by 1/den, + bias' + residual
                pj_ps = post_ps.tile([128, 512], FP, tag="post")
                nc.tensor.matmul(out=pj_ps[0:C, :], lhsT=wpT[:], rhs=av_sb[:])
                o_sb = outp.tile([C, NT], FP, tag="o_sb")
                nc.vector.tensor_mul(out=o_sb[:], in0=pj_ps[0:C, :], in1=dbc[:])
                nc.vector.scalar_tensor_tensor(
                    out=o_sb[:], in0=o_sb[:], scalar=bpp[:], in1=x_sb[:, nsl],
                    op0=mybir.AluOpType.add, op1=mybir.AluOpType.add)
                nc.sync.dma_start(out=out_d[:, nsl], in_=o_sb[:])

            for nt in range(N_NT + 1):
                if nt < N_NT:
                    emit_scores(nt)
                if nt > 0:
                    emit_av_post(nt - 1)

    nc.finalize()  # Bacc.finalize runs the wait-splitting legalization
    return nc


_cached = {}


def _install_trace_hook():
    """The agent image lacks antenv.axon_hooks, so run_bass_kernel_spmd's
    trace path degrades. Recreate the module + NTFF hook locally."""
    import sys, types
    import antenv
    if "antenv.axon_hooks" in sys.modules:
        return
    mod = types.ModuleType("antenv.axon_hooks")
    holder = {"hook": None}
    mod.set_axon_ntff_profile_hook = lambda h: holder.__setitem__("hook", h)
    mod.get_axon_ntff_profile_hook = lambda: holder["hook"]
    sys.modules["antenv.axon_hooks"] = mod
    antenv.axon_hooks = mod
    from trn_agent_boot.trn_boot import _ntff_profile_via_ctypes
    mod.set_axon_ntff_profile_hook(_ntff_profile_via_ctypes("/opt/axon/libaxon_pjrt.so"))
    import concourse.bass_utils as bu
    bu.upload_artifacts = lambda tmpdir: tmpdir


def make_consts(Wq, bq, Wk, Wv, bv, Wp, bp, gn_w, gn_b):
    f32 = lambda a: np.ascontiguousarray(np.asarray(a, np.float32))
    scale = np.float32(1.0 / np.sqrt(np.float32(C)))
    gmask = np.zeros((C, G), np.float32)
    gbcast = np.zeros((G, C), np.float32)
    for g in range(G):
        gmask[g * 8:(g + 1) * 8, g] = 1.0 / 8.0
        gbcast[g, g * 8:(g + 1) * 8] = 1.0
    wqT = np.asarray(Wq, np.float32).T * scale
    wkT = np.asarray(Wk, np.float32).T
    return {
        "wqT2": f32(np.concatenate([wqT, wqT], axis=1)),
        "wkT2": f32(np.concatenate([wkT, wkT], axis=1)),
        "bq2": f32(np.tile(np.asarray(bq, np.float32) * scale, 2))[:, None],
        "wvT": f32(np.asarray(Wv).T),
        "wpT": f32(np.asarray(Wp).T),
        "bpp": f32(np.asarray(bp) + np.asarray(Wp) @ np.asarray(bv))[:, None],
        "gamma": f32(gn_w)[:, None],
        "beta": f32(gn_b)[:, None],
        "gmask": gmask,
        "gbcast": gbcast,
    }


def kernel(x, gn_w, gn_b, Wq, bq, Wk, bk, Wv, bv, Wp, bp, _trace=False):
    x = np.ascontiguousarray(np.asarray(x, np.float32)).reshape(B, C, N)
    consts = make_consts(Wq, bq, Wk, Wv, bv, Wp, bp, gn_w, gn_b)

    if _trace:
        _install_trace_hook()

    if "nc" not in _cached:
        _cached["nc"] = build_program()
    nc = _cached["nc"]

    in_maps = [dict(consts, x=np.ascontiguousarray(x[i])) for i in range(B)]
    res = run_bass_kernel_spmd(nc, in_maps, core_ids=list(range(B)), trace=_trace)
    last_run_info["exec_time_ns"] = res.exec_time_ns
    last_run_info["mean_exec_time_ns"] = res.mean_exec_time_ns
    out = np.stack([res.results[i]["out"] for i in range(B)], axis=0)
    return out.reshape(B, C, H, W)
